# revision 1
# baseline (speedup 1.0000x reference)
"""Bahdanau attention kernel for 8 Trainium2 NeuronCores.

Problem (hardcoded shapes): B=32, T=8192, D_ENC=256, D_HID=512, D_ATT=512.
    proj = encoder_out @ w1 + b1 + (h @ w2 + b2) + (c @ w3 + b3)   # [B,T,512]
    scores = tanh(proj) @ wv (+ bv)                                # [B,T,1]
    attn = softmax(scores, axis=T)
    context = sum_t attn * encoder_out                             # [B,256]

Sharding: data-parallel over batch, 4 batches per core, no collectives.

Device strategy (per core, per batch):
  - encoder_out is fed twice in bf16 (transposed [256,8192] for the
    projection matmul; natural [8192,257] with an appended ones column for
    the context accumulation) — the two bf16 copies cost the same HBM
    traffic as one f32 copy.
  - Pass A (8 chunks of 1024 timesteps): hidden^T[j] = w1[k,j]^T @ encT
    accumulated over k in PSUM; tanh with the per-batch bias fused as the
    ACT per-partition bias (one FD=1024 instruction per j);
    scores = sum_j wv_j^T @ tanh_j on PE (M=1); score rows are staged to
    SBUF and dropped into a per-half-batch [4, 1024] S tile by a tiny DMA.
  - Per half-batch: 8 PE transposes ([4,128] blocks against a host-fed
    identity) turn score rows into column form in PSUM; one ACT exp
    produces e in bf16.  Scores are O(1) so no max subtraction is needed;
    a constant shift cancels in softmax anyway, which is also why bv is
    dropped.
  - Pass B: per 128 timesteps, acc += e_col * encN via the fused
    scalar_tensor_tensor op on VectorE (two interleaved accumulators).
    The ones column of encN makes the same op accumulate Z = sum(e).
    Finally ctx/Z via a ones^T @ acc matmul, reciprocal, and scale.
    The very last half-batch accumulates on the (otherwise idle) PE
    instead, shrinking the end-of-kernel tail.
  Pass B lags pass A by half a batch, so the end-of-kernel exposed tail is
  only half a batch of accumulation work.
"""

import os
import sys

for _p in ("/opt/trn_rl_repo", "/root/.axon_site", "/root/.axon_site/_ro/pypackages"):
    if os.path.isdir(_p) and _p not in sys.path:
        sys.path.append(_p)

import numpy as np
import ml_dtypes

import concourse.bass as bass
import concourse.tile as tile
from concourse import bacc, bass_isa, mybir
from concourse.bass_utils import run_bass_kernel_spmd

BF16 = ml_dtypes.bfloat16

B, T, D_ENC, D_HID, D_ATT = 32, 8192, 256, 512, 512
N_CORES = 8
BPC = B // N_CORES          # batches per core = 4
P = 128                     # partitions
TC = 1024                   # pass-A chunk (timesteps)
HTC = TC // 2               # matmul moving-dim half = 512
NCH = T // TC               # pass-A chunks per batch = 8
HCH = NCH // 2              # chunks per half-batch = 4
NU = TC // P                # 128-blocks per chunk = 8
NCOL = T // P               # score columns per batch = 64
KD = D_ENC // P             # k-tiles of the contraction dim = 2
NJ = D_ATT // P             # a-tiles = 4
DE1 = D_ENC + 1             # encN row with ones column = 257

_PROGRAM_CACHE = {}


def _build_program():
    """Build and finalize the SPMD program (identical on all 8 cores)."""
    if "nc" in _PROGRAM_CACHE:
        return _PROGRAM_CACHE["nc"]

    f32 = mybir.dt.float32
    bf16 = mybir.dt.bfloat16
    Act = mybir.ActivationFunctionType
    Alu = mybir.AluOpType

    nc = bacc.Bacc("TRN2", target_bir_lowering=False, debug=False,
                   num_devices=N_CORES)

    encT = nc.dram_tensor("encT", [BPC, D_ENC, T], bf16, kind="ExternalInput")
    encN = nc.dram_tensor("encN", [BPC, T, DE1], bf16, kind="ExternalInput")
    w1t = nc.dram_tensor("w1t", [P, KD, NJ, P], bf16, kind="ExternalInput")
    wvt = nc.dram_tensor("wvt", [P, NJ], bf16, kind="ExternalInput")
    vbt = nc.dram_tensor("vbt", [P, BPC * NJ], f32, kind="ExternalInput")
    outd = nc.dram_tensor("out", [BPC, D_ENC], f32, kind="ExternalOutput")
    sscr = nc.dram_tensor("sscr", [BPC * NCH, TC], mybir.dt.bfloat16)

    with tile.TileContext(nc) as tc:
        import contextlib
        with contextlib.ExitStack() as ctx:
            const = ctx.enter_context(tc.tile_pool(name="const", bufs=1))
            encT_pool = ctx.enter_context(tc.tile_pool(name="encT", bufs=6))
            encN_pool = ctx.enter_context(tc.tile_pool(name="encN", bufs=4))
            tanh_pool = ctx.enter_context(tc.tile_pool(name="tanh", bufs=12))
            ssb_pool = ctx.enter_context(tc.tile_pool(name="ssb", bufs=4))
            e_pool = ctx.enter_context(tc.tile_pool(name="e", bufs=2))
            sm_pool = ctx.enter_context(tc.tile_pool(name="sm", bufs=4))
            osb_pool = ctx.enter_context(tc.tile_pool(name="osb", bufs=2))
            accv_pool = ctx.enter_context(tc.tile_pool(name="accv", bufs=2))
            accg_pool = ctx.enter_context(tc.tile_pool(name="accg", bufs=2))
            hid_psum = ctx.enter_context(
                tc.tile_pool(name="hid", bufs=2, space="PSUM"))
            sc_psum = ctx.enter_context(
                tc.tile_pool(name="sc", bufs=1, space="PSUM"))
            epc_pool = ctx.enter_context(tc.tile_pool(name="epc", bufs=10))
            cf_psum = ctx.enter_context(
                tc.tile_pool(name="cfin", bufs=1, space="PSUM"))

            # constants
            w1_sb = const.tile([P, KD, NJ, P], bf16)
            nc.scalar.dma_start(w1_sb[:], w1t[:])
            wvt_sb = const.tile([P, NJ], bf16)
            nc.scalar.dma_start(wvt_sb[:], wvt[:])
            vbt_sb = const.tile([P, BPC * NJ], f32)
            nc.scalar.dma_start(vbt_sb[:], vbt[:])
            ones128 = const.tile([P, 1], f32)
            nc.gpsimd.memset(ones128[:], 1.0)

            epc_of = {}  # chunk -> [128, 8] bf16 SBUF score-column tile
            e_sb = {}    # per-batch [128, 64] bf16: exp(scores)
            acc_v = {}   # per-batch [128, 257] f32: DVE accumulator (even g)
            acc_g = {}   # per-batch [128, 257] f32: DVE accumulator (odd g)
            cfp = {}     # last batch: PE-accumulated [1, 257] psum

            tanh_of = {}   # chunk index -> list of tanh tiles

            def emit_A_main(b, i):
                encT_t = encT_pool.tile([P, KD, TC], bf16)
                nc.sync.dma_start(
                    encT_t[:],
                    encT[b, :, i * TC:(i + 1) * TC]
                        .rearrange("(k p) t -> p k t", p=P))
                tanh_tiles = []
                for j in range(NJ):
                    h_ps = hid_psum.tile([P, TC], f32, tag="hid")
                    for k in range(KD):
                        for h in range(2):
                            nc.tensor.matmul(
                                h_ps[:, h * HTC:(h + 1) * HTC],
                                w1_sb[:, k, j, :],
                                encT_t[:, k, h * HTC:(h + 1) * HTC],
                                start=(k == 0), stop=(k == KD - 1))
                    th = tanh_pool.tile([P, TC], bf16, tag="tanh")
                    nc.scalar.activation(
                        th[:], h_ps[:], Act.Tanh,
                        bias=vbt_sb[:, b * NJ + j: b * NJ + j + 1])
                    tanh_tiles.append(th)
                tanh_of[i] = tanh_tiles

            def emit_A_scores(b, i):
                tanh_tiles = tanh_of.pop(i)
                s_ps = sc_psum.tile([1, TC], f32, tag="sc")
                for j in range(NJ):
                    for h in range(2):
                        nc.tensor.matmul(
                            s_ps[:, h * HTC:(h + 1) * HTC],
                            wvt_sb[:, j:j + 1],
                            tanh_tiles[j][:, h * HTC:(h + 1) * HTC],
                            start=(j == 0), stop=(j == NJ - 1))
                # Engine APs must start at a 32-aligned partition, so the
                # score row is staged at partition 0; a strided SBUF->SBUF
                # scatter-DMA (HWDGE) then drops it directly into column
                # form: columns m = i*8+u hold scores for t = m*128 + p.
                s_sb = ssb_pool.tile([1, TC], bf16, tag="ssb")
                nc.vector.tensor_copy(s_sb[:], s_ps[:])
                row = sscr[b * NCH + i: b * NCH + i + 1, :]
                nc.sync.dma_start(row, s_sb[:])
                epc = epc_pool.tile([P, NU], bf16, tag="epc")
                nc.sync.dma_start(
                    epc[:], row.rearrange("o (u p) -> p (o u)", p=P))
                epc_of[i] = epc

            def emit_half_epilogue(b, half):
                for c in range(half * HCH, (half + 1) * HCH):
                    nc.scalar.activation(
                        e_sb[b][:, c * NU:(c + 1) * NU],
                        epc_of.pop(c)[:], Act.Exp)
                if half == 0:
                    acc_v[b] = accv_pool.tile([P, DE1], f32, tag="accv",
                                              name=f"acc_v{b}")
                    nc.gpsimd.memset(acc_v[b][:], 0.0)
                    acc_g[b] = accg_pool.tile([P, DE1], f32, tag="accg",
                                              name=f"acc_g{b}")
                    nc.gpsimd.memset(acc_g[b][:], 0.0)

            def emit_B_group(b, g):
                """One pass-B group = super-chunk g (1024 timesteps)."""
                encN_t = encN_pool.tile([P, NU, DE1], bf16)
                nc.sync.dma_start(
                    encN_t[:],
                    encN[b, g * TC:(g + 1) * TC, :]
                        .rearrange("(n p) d -> p n d", p=P))
                if b == BPC - 1 and g >= HCH:
                    # PE is otherwise idle in the kernel tail: accumulate
                    # this half directly in PSUM via matmuls.
                    if g == HCH:
                        cfp["t"] = cf_psum.tile([1, DE1], f32, tag="cfin",
                                                name="cfp_last")
                    for n in range(NU):
                        m = NU * g + n
                        nc.tensor.matmul(
                            cfp["t"][:],
                            e_sb[b][:, m:m + 1],
                            encN_t[:, n, :],
                            start=(g == HCH and n == 0), stop=False)
                    return
                acc = acc_v if g % 2 == 0 else acc_g
                for n in range(NU):
                    m = NU * g + n
                    nc.vector.scalar_tensor_tensor(
                        acc[b][:], encN_t[:, n, :],
                        e_sb[b][:, m:m + 1],
                        acc[b][:],
                        op0=Alu.mult, op1=Alu.add)

            def emit_B_finalize(b):
                if b == BPC - 1:
                    cf = cfp["t"]
                    nc.tensor.matmul(cf[:], ones128[:], acc_v[b][:],
                                     start=False, stop=False)
                    nc.tensor.matmul(cf[:], ones128[:], acc_g[b][:],
                                     start=False, stop=True)
                else:
                    cf = cf_psum.tile([1, DE1], f32, tag="cfin")
                    nc.tensor.matmul(cf[:], ones128[:], acc_v[b][:],
                                     start=True, stop=False)
                    nc.tensor.matmul(cf[:], ones128[:], acc_g[b][:],
                                     start=False, stop=True)
                rzb = sm_pool.tile([1, 1], f32, tag="rz", name=f"rz{b}")
                nc.vector.reciprocal(rzb[:], cf[:, D_ENC:D_ENC + 1])
                o_sb = osb_pool.tile([1, D_ENC], f32, tag="osb")
                nc.vector.tensor_scalar_mul(o_sb[:], cf[:, 0:D_ENC], rzb[:])
                nc.sync.dma_start(outd[b:b + 1, :], o_sb[:])

            for step in range(BPC + 1):
                if step < BPC:
                    e_sb[step] = e_pool.tile([P, NCOL], bf16, tag="e",
                                             name=f"e_sb{step}")
                for i in range(NCH):
                    if step < BPC:
                        if i > 0:
                            emit_A_scores(step, i - 1)
                        emit_A_main(step, i)
                        if i == HCH:
                            emit_half_epilogue(step, 0)
                    if i < HCH:
                        if step >= 1:
                            emit_B_group(step - 1, HCH + i)
                            if i == HCH - 1:
                                emit_B_finalize(step - 1)
                    else:
                        if step < BPC:
                            emit_B_group(step, i - HCH)
                if step < BPC:
                    emit_A_scores(step, NCH - 1)
                    emit_half_epilogue(step, 1)

    nc.finalize()
    _PROGRAM_CACHE["nc"] = nc
    return nc


def _prep_inputs(encoder_out, hidden_state_h, hidden_state_c,
                 w1, b1, w2, b2, w3, b3, wv, bv):
    """Host-side sharding + layout prep. Returns per-core input maps."""
    enc = np.asarray(encoder_out, dtype=np.float32)
    # per-batch bias vector: b1 + h@w2 + b2 + c@w3 + b3  (tiny, exact f32)
    vb = (np.asarray(b1, np.float32)
          + np.asarray(hidden_state_h, np.float32) @ np.asarray(w2, np.float32)
          + np.asarray(b2, np.float32)
          + np.asarray(hidden_state_c, np.float32) @ np.asarray(w3, np.float32)
          + np.asarray(b3, np.float32))                        # [B, D_ATT]
    # bv shifts every score equally -> cancels in softmax; dropped.

    w1_h = np.ascontiguousarray(
        np.asarray(w1, np.float32).reshape(KD, P, NJ, P).transpose(1, 0, 2, 3)
    ).astype(BF16)                                             # [128,2,4,128]
    wv_h = np.ascontiguousarray(
        np.asarray(wv, np.float32).reshape(NJ, P).T).astype(BF16)  # [128,4]
    ident_h = np.eye(HCH, dtype=np.float32).astype(BF16)

    in_maps = []
    for c in range(N_CORES):
        sl = slice(c * BPC, (c + 1) * BPC)
        enc_c = enc[sl]                                        # [4, T, 256]
        encT_c = np.ascontiguousarray(enc_c.transpose(0, 2, 1)).astype(BF16)
        encN_c = np.ascontiguousarray(np.concatenate(
            [enc_c, np.ones((BPC, T, 1), np.float32)], axis=2)).astype(BF16)
        vbt_c = np.ascontiguousarray(
            vb[sl].reshape(BPC, NJ, P).transpose(2, 0, 1).reshape(P, BPC * NJ)
        ).astype(np.float32)
        in_maps.append({
            "encT": encT_c,
            "encN": encN_c,
            "w1t": w1_h,
            "wvt": wv_h,
            "vbt": vbt_c,
            "ident": ident_h,
        })
    return in_maps


def kernel(**inputs):
    nc = _build_program()
    in_maps = _prep_inputs(**inputs)
    res = run_bass_kernel_spmd(nc, in_maps, list(range(N_CORES)))
    out = np.concatenate([res.results[c]["out"] for c in range(N_CORES)],
                         axis=0)
    return out.astype(np.float32)


if __name__ == "__main__":
    rng = np.random.default_rng(0)
    ins = {
        "encoder_out": rng.standard_normal((B, T, D_ENC), dtype=np.float32),
        "hidden_state_h": rng.standard_normal((B, D_HID), dtype=np.float32),
        "hidden_state_c": rng.standard_normal((B, D_HID), dtype=np.float32),
        "w1": (rng.standard_normal((D_ENC, D_ATT), dtype=np.float32)
               / np.sqrt(D_ENC)),
        "b1": np.zeros(D_ATT, np.float32),
        "w2": (rng.standard_normal((D_HID, D_ATT), dtype=np.float32)
               / np.sqrt(D_HID)),
        "b2": np.zeros(D_ATT, np.float32),
        "w3": (rng.standard_normal((D_HID, D_ATT), dtype=np.float32)
               / np.sqrt(D_HID)),
        "b3": np.zeros(D_ATT, np.float32),
        "wv": (rng.standard_normal((D_ATT, 1), dtype=np.float32)
               / np.sqrt(D_ATT)),
        "bv": np.zeros(1, np.float32),
    }
    got = kernel(**ins)
    print("kernel output:", got.shape, got.dtype)



# revision 5
# speedup vs baseline: 1.2251x; 1.2251x over previous
"""Bahdanau attention kernel for 8 Trainium2 NeuronCores.

Problem (hardcoded shapes): B=32, T=8192, D_ENC=256, D_HID=512, D_ATT=512.
    proj = encoder_out @ w1 + b1 + (h @ w2 + b2) + (c @ w3 + b3)   # [B,T,512]
    scores = tanh(proj) @ wv (+ bv)                                # [B,T,1]
    attn = softmax(scores, axis=T)
    context = sum_t attn * encoder_out                             # [B,256]

Sharding: data-parallel over batch, 4 batches per core, no collectives.

Device strategy (per core, per batch):
  - encoder_out is fed twice in bf16 (transposed [256,8192] for the
    projection matmul; natural [8192,257] with an appended ones column for
    the context accumulation) — the two bf16 copies cost the same HBM
    traffic as one f32 copy.
  - Per chunk of 1024 timesteps: hidden^T[j] = w1[k,j]^T @ encT in PSUM;
    tanh with the per-batch bias fused as the ACT per-partition bias.
  - Scores use PE column tiling: stationary = wv_j broadcast to 32
    columns, 4 matmuls per j land in col-groups (0,32,64,96) covering the
    four 256-timestep quarters of the chunk concurrently; PSUM accumulates
    over j.  One DVE copy stages the psum rows to SBUF; a strided
    SBUF->SBUF scatter-DMA drops them into per-batch column form
    [128, 64]; one EXP per batch produces e (scores are O(1), so no max
    subtraction; constant bv cancels in softmax and is dropped).
  - Pass B runs on the otherwise-idle PE via the same column tiling:
    ctx_partial[q] += e_col[m]^T @ encN_block[m] for m%4==q, accumulated
    in one PSUM bank across the whole batch; the ones column of encN
    accumulates Z = sum(e) for free.  Finalize: 3 DVE adds across the 4
    col-group rows, reciprocal of Z, scale, DMA out.
  Pass B for batch b is interleaved into batch b+1's chunk loop (1-batch
  lag), so only the last batch's pass B is an exposed tail.
"""

import os
import sys

for _p in ("/opt/trn_rl_repo", "/root/.axon_site", "/root/.axon_site/_ro/pypackages"):
    if os.path.isdir(_p) and _p not in sys.path:
        sys.path.append(_p)

import numpy as np
import ml_dtypes

import concourse.bass as bass
import concourse.tile as tile
from concourse import bacc, bass_isa, mybir
from concourse.bass_utils import run_bass_kernel_spmd

BF16 = ml_dtypes.bfloat16

B, T, D_ENC, D_HID, D_ATT = 32, 8192, 256, 512, 512
N_CORES = 8
BPC = B // N_CORES          # batches per core = 4
P = 128                     # partitions
TC = 1024                   # chunk (timesteps)
HTC = 512                   # matmul moving-dim half
QTC = 256                   # chunk quarter (score col-tile streams)
NCH = T // TC               # chunks per batch = 8
NU = TC // P                # 128-blocks per chunk = 8
NCOL = T // P               # e-columns per batch = 64
KD = D_ENC // P             # k-tiles of the contraction dim = 2
NJ = D_ATT // P             # a-tiles = 4
DE1 = D_ENC + 1             # encN row with ones column = 257

_PROGRAM_CACHE = {}


def _build_program():
    """Build and finalize the SPMD program (identical on all 8 cores)."""
    if "nc" in _PROGRAM_CACHE:
        return _PROGRAM_CACHE["nc"]

    f32 = mybir.dt.float32
    bf16 = mybir.dt.bfloat16
    Act = mybir.ActivationFunctionType

    nc = bacc.Bacc("TRN2", target_bir_lowering=False, debug=False,
                   num_devices=N_CORES)

    encT = nc.dram_tensor("encT", [BPC, D_ENC, T], bf16, kind="ExternalInput")
    encN = nc.dram_tensor("encN", [BPC, T, DE1], bf16, kind="ExternalInput")
    w1t = nc.dram_tensor("w1t", [P, KD, NJ, P], bf16, kind="ExternalInput")
    wvb = nc.dram_tensor("wvb", [P, NJ, 32], bf16, kind="ExternalInput")
    vbt = nc.dram_tensor("vbt", [P, BPC * NJ], f32, kind="ExternalInput")
    outd = nc.dram_tensor("out", [BPC, D_ENC], f32, kind="ExternalOutput")
    scr = nc.dram_tensor("scr", [BPC, NCH, 4, QTC], bf16)

    with tile.TileContext(nc) as tc:
        import contextlib
        with contextlib.ExitStack() as ctx:
            const = ctx.enter_context(tc.tile_pool(name="const", bufs=1))
            encT_pool = ctx.enter_context(tc.tile_pool(name="encT", bufs=4))
            encN_pool = ctx.enter_context(tc.tile_pool(name="encN", bufs=4))
            tanh_pool = ctx.enter_context(tc.tile_pool(name="tanh", bufs=10))
            cp_pool = ctx.enter_context(tc.tile_pool(name="cp", bufs=2))
            scoL_pool = ctx.enter_context(tc.tile_pool(name="scoL", bufs=2))
            e_pool = ctx.enter_context(tc.tile_pool(name="e", bufs=2))
            fin_pool = ctx.enter_context(tc.tile_pool(name="fin", bufs=2))
            osb_pool = ctx.enter_context(tc.tile_pool(name="osb", bufs=2))
            hid_psum = ctx.enter_context(
                tc.tile_pool(name="hid", bufs=3, space="PSUM"))
            sc_psum = ctx.enter_context(
                tc.tile_pool(name="sc", bufs=1, space="PSUM"))
            cf_psum = ctx.enter_context(
                tc.tile_pool(name="cf", bufs=1, space="PSUM"))

            # constants
            w1_sb = const.tile([P, KD, NJ, P], bf16)
            nc.scalar.dma_start(w1_sb[:], w1t[:])
            wvb_sb = const.tile([P, NJ, 32], bf16)
            nc.scalar.dma_start(wvb_sb[:], wvb[:])
            vbt_sb = const.tile([P, BPC * NJ], f32)
            nc.scalar.dma_start(vbt_sb[:], vbt[:])

            tanh_of = {}   # (chunk) -> list of 4 tanh tiles
            scoL_of = {}   # batch -> [128, 64] bf16 columnized scores
            e_of = {}      # batch -> [128, 64] bf16 exp(scores)
            cf_of = {}     # batch -> [128, 257] psum ctx accumulator

            def emit_proj(b, i):
                encT_t = encT_pool.tile([P, KD, TC], bf16)
                nc.sync.dma_start(
                    encT_t[:],
                    encT[b, :, i * TC:(i + 1) * TC]
                        .rearrange("(k p) t -> p k t", p=P))
                tanh_tiles = []
                for j in range(NJ):
                    h_ps = hid_psum.tile([P, TC], f32, tag="hid")
                    for k in range(KD):
                        for h in range(2):
                            nc.tensor.matmul(
                                h_ps[:, h * HTC:(h + 1) * HTC],
                                w1_sb[:, k, j, :],
                                encT_t[:, k, h * HTC:(h + 1) * HTC],
                                start=(k == 0), stop=(k == KD - 1))
                    th = tanh_pool.tile([P, TC], bf16, tag="tanh")
                    nc.scalar.activation(
                        th[:], h_ps[:], Act.Tanh,
                        bias=vbt_sb[:, b * NJ + j: b * NJ + j + 1])
                    tanh_tiles.append(th)
                tanh_of[i] = tanh_tiles

            def emit_scores(b, i):
                tanh_tiles = tanh_of.pop(i)
                s_ps = sc_psum.tile([P, QTC], f32, tag="sc")
                for j in range(NJ):
                    th = tanh_tiles[j]
                    for q in range(4):
                        nc.tensor.matmul(
                            s_ps[32 * q:32 * q + 32, :],
                            wvb_sb[:, j, :],
                            th[:, q * QTC:(q + 1) * QTC],
                            start=(j == 0), stop=(j == NJ - 1),
                            tile_position=(0, 32 * q))
                cp = cp_pool.tile([P, QTC], bf16, tag="cp")
                nc.vector.tensor_copy(cp[:], s_ps[:])
                # Bounce the 4 score rows (partitions 0/32/64/96) through
                # DRAM into column form: col c = i*8 + q*2 + u holds scores
                # for t = i*1024 + (q*2+u)*128 + p.
                row = scr[b, i]
                nc.sync.dma_start(row, cp[0:97:32, :])
                nc.sync.dma_start(
                    scoL_of[b][:, i * NU:(i + 1) * NU],
                    row.rearrange("q (u p) -> p (q u)", p=P))

            def emit_exp(b):
                e = e_pool.tile([P, NCOL], bf16, tag="e", name=f"e{b}")
                nc.scalar.activation(e[:], scoL_of.pop(b)[:], Act.Exp)
                e_of[b] = e

            def emit_passB(b, i):
                encN_t = encN_pool.tile([P, NU, DE1], bf16)
                nc.sync.dma_start(
                    encN_t[:],
                    encN[b, i * TC:(i + 1) * TC, :]
                        .rearrange("(n p) d -> p n d", p=P))
                if i == 0:
                    cf_of[b] = cf_psum.tile([P, DE1], f32, tag="cf",
                                            name=f"cf{b}")
                cf = cf_of[b]
                for n in range(NU):
                    m = NU * i + n
                    q = n % 4
                    nc.tensor.matmul(
                        cf[32 * q:32 * q + 1, :],
                        e_of[b][:, m:m + 1],
                        encN_t[:, n, :],
                        start=(i == 0 and n < 4),
                        stop=(i == NCH - 1 and n >= NU - 4),
                        tile_position=(0, 32 * q))

            def emit_finalize(b):
                # DVE may read at most one PSUM operand per instruction:
                # copy row 0 out, then chain in-place adds of rows 32/64/96.
                cf = cf_of.pop(b)
                t0 = fin_pool.tile([1, DE1], f32, tag="t0")
                nc.vector.tensor_copy(t0[:], cf[0:1, :])
                for q in range(1, 4):
                    nc.vector.tensor_add(t0[:], t0[:], cf[32 * q:32 * q + 1, :])
                rz = fin_pool.tile([1, 1], f32, tag="rz")
                nc.vector.reciprocal(rz[:], t0[:, D_ENC:D_ENC + 1])
                o_sb = osb_pool.tile([1, D_ENC], f32, tag="osb")
                nc.vector.tensor_scalar_mul(o_sb[:], t0[:, 0:D_ENC], rz[:])
                nc.sync.dma_start(outd[b:b + 1, :], o_sb[:])

            for b in range(BPC):
                scoL_of[b] = scoL_pool.tile([P, NCOL], bf16, tag="scoL",
                                            name=f"scoL{b}")
                for i in range(NCH):
                    emit_proj(b, i)
                    if i > 0:
                        emit_scores(b, i - 1)
                    if b > 0:
                        emit_passB(b - 1, i)
                        if i == NCH - 1:
                            emit_finalize(b - 1)
                emit_scores(b, NCH - 1)
                emit_exp(b)
            for i in range(NCH):
                emit_passB(BPC - 1, i)
            emit_finalize(BPC - 1)

    nc.finalize()
    _PROGRAM_CACHE["nc"] = nc
    return nc


def _prep_inputs(encoder_out, hidden_state_h, hidden_state_c,
                 w1, b1, w2, b2, w3, b3, wv, bv):
    """Host-side sharding + layout prep. Returns per-core input maps."""
    enc = np.asarray(encoder_out, dtype=np.float32)
    # per-batch bias vector: b1 + h@w2 + b2 + c@w3 + b3  (tiny, exact f32)
    vb = (np.asarray(b1, np.float32)
          + np.asarray(hidden_state_h, np.float32) @ np.asarray(w2, np.float32)
          + np.asarray(b2, np.float32)
          + np.asarray(hidden_state_c, np.float32) @ np.asarray(w3, np.float32)
          + np.asarray(b3, np.float32))                        # [B, D_ATT]
    # bv shifts every score equally -> cancels in softmax; dropped.

    w1_h = np.ascontiguousarray(
        np.asarray(w1, np.float32).reshape(KD, P, NJ, P).transpose(1, 0, 2, 3)
    ).astype(BF16)                                             # [128,2,4,128]
    wv_b = np.ascontiguousarray(np.broadcast_to(
        np.asarray(wv, np.float32).reshape(NJ, P).transpose(1, 0)[:, :, None],
        (P, NJ, 32))).astype(BF16)                             # [128,4,32]

    in_maps = []
    for c in range(N_CORES):
        sl = slice(c * BPC, (c + 1) * BPC)
        enc_c = enc[sl]                                        # [4, T, 256]
        encT_c = np.ascontiguousarray(enc_c.transpose(0, 2, 1)).astype(BF16)
        encN_c = np.ascontiguousarray(np.concatenate(
            [enc_c, np.ones((BPC, T, 1), np.float32)], axis=2)).astype(BF16)
        vbt_c = np.ascontiguousarray(
            vb[sl].reshape(BPC, NJ, P).transpose(2, 0, 1).reshape(P, BPC * NJ)
        ).astype(np.float32)
        in_maps.append({
            "encT": encT_c,
            "encN": encN_c,
            "w1t": w1_h,
            "wvb": wv_b,
            "vbt": vbt_c,
        })
    return in_maps


def kernel(**inputs):
    nc = _build_program()
    in_maps = _prep_inputs(**inputs)
    res = run_bass_kernel_spmd(nc, in_maps, list(range(N_CORES)))
    out = np.concatenate([res.results[c]["out"] for c in range(N_CORES)],
                         axis=0)
    return out.astype(np.float32)


if __name__ == "__main__":
    rng = np.random.default_rng(0)
    ins = {
        "encoder_out": rng.standard_normal((B, T, D_ENC), dtype=np.float32),
        "hidden_state_h": rng.standard_normal((B, D_HID), dtype=np.float32),
        "hidden_state_c": rng.standard_normal((B, D_HID), dtype=np.float32),
        "w1": (rng.standard_normal((D_ENC, D_ATT), dtype=np.float32)
               / np.sqrt(D_ENC)),
        "b1": np.zeros(D_ATT, np.float32),
        "w2": (rng.standard_normal((D_HID, D_ATT), dtype=np.float32)
               / np.sqrt(D_HID)),
        "b2": np.zeros(D_ATT, np.float32),
        "w3": (rng.standard_normal((D_HID, D_ATT), dtype=np.float32)
               / np.sqrt(D_HID)),
        "b3": np.zeros(D_ATT, np.float32),
        "wv": (rng.standard_normal((D_ATT, 1), dtype=np.float32)
               / np.sqrt(D_ATT)),
        "bv": np.zeros(1, np.float32),
    }
    got = kernel(**ins)
    print("kernel output:", got.shape, got.dtype)
